# revision 43
# baseline (speedup 1.0000x reference)
"""Trainium2 Bass kernel for SAGAN-style self-attention (nn_Attention_13056700580138).

Reference computation (per batch element, N = H*W = 4096, C = 256, CK = 32):
    f  = x @ Wf + bf            [N, CK]
    g  = x @ Wg + bg            [N, CK]
    hh = x @ Wh + bh            [N, C]
    S  = g @ f^T                [N, N]
    A  = softmax(S, axis=-1)
    o  = A @ hh                 [N, C]
    out = gamma * (o @ Wo + bo) + x

Sharding: data-parallel over batch - one batch element per NeuronCore (B = 8).

Per-core structure (v2 — rebuilt around the TimelineSim cost model):
  * All matmuls float32r (FP22 reads): 1 cycle/row at moving-dim >= 256.
  * Output projection folded: (A@hh)@Wo + bo = A@(x@(Wh@Wo)) + (bh@Wo + bo)
    (softmax rows sum to one, so the row bias passes through exactly).
  * Scores computed transposed (S^T tiles [128 keys, 512 queries]) so the
    exp'd tiles feed the A @ hw accumulation directly as stationary operands.
    Key block (4g+t) lives in tile_position row group t; attention iteration
    g consumes exactly pixel slice g, so the whole prologue pipelines
    slice-by-slice with no assembly barrier.
  * f^T/g^T are replicated into all 4 PE row groups by a single REP-matmul
    (block-diagonal selector) per projection instead of SBUF-rearrange DMAs.
  * Softmax needs no max subtraction (|scores| < ~60, exp fits fp32).  Row
    sums come from near-free N=1 matmuls (same stationary exp tile, ones
    column) into a dedicated PSUM bank, so the o accumulators pack two
    128-query blocks per PSUM bank ([128, 512]) and rotate through 3 banks —
    query-slice epilogues overlap the next slice's accumulation.
  * PSUM budget: 4 score banks + 3 o banks + 1 row-sum bank = 8.
  * Epilogue per 128-pixel block: reciprocal + gamma, then one fused
    multiply-add against the fp32 residual, DMA out per 256-pixel pair.
"""

from contextlib import ExitStack

import numpy as np

import bass_rust
import concourse.bass as bass
import concourse.mybir as mybir
import concourse.tile as tile
from concourse.bass_utils import run_bass_kernel_spmd
from concourse.masks import make_identity
from concourse.vector_clock import ScopedClock

FP = mybir.dt.float32
FPR = mybir.dt.float32r
BF = mybir.dt.bfloat16
AF = mybir.ActivationFunctionType
ALU = mybir.AluOpType

B, H, W, C = 8, 64, 64, 256
CK = C // 8
N = H * W  # 4096
NCORES = 8


# --- workaround: walrus in this toolchain lowers at most one sync-wait per SP
# CTRL instruction, but TileContext's final drain carries one wait per busy
# processor. Split them across single-wait carrier nops (same engine queue,
# program order => identical semantics).
def _split_drain_and_barrier(self, tick_clock, wait_clock):
    nc = self.nc
    ticks = list(eval(repr(tick_clock.global_clock).replace("VectorClock", "")))
    nproc = len(ticks)
    for i, t in enumerate(ticks):
        if t > 0:
            sub = [0] * nproc
            sub[i] = t
            carrier = nc.sync.nop(nofuse=True, hint="drain_split_wait")
            wait_clock.add_sem_waits(
                carrier.ins, ScopedClock({None: bass_rust.VectorClock(sub)})
            )
    nc.sync.drain()
    nc.all_engine_barrier()
    assert self.sems is not None
    popped = nc._tile_sem_poison_stack.pop()
    assert popped is self._sem_poison
    nc.clear_and_free_semaphores(list(self.sems.allocated().values()))
    nc.all_engine_barrier()


tile.TileContext._drain_and_barrier = _split_drain_and_barrier


def _split_instruction_waits(nc):
    """walrus in this toolchain lowers at most one sync-wait per instruction
    for several instruction templates. After Tile scheduling, move any extra
    waits onto single-wait carrier nops inserted just before the instruction
    on the same engine queue (identical blocking semantics)."""
    cnt = 0
    for fn in nc.m.functions:
        for bb in fn.blocks:
            out = []
            changed = False
            for ins in bb.instructions:
                si = ins.sync_info
                waits = list(si.on_wait) if (si is not None and si.on_wait) else []
                if len(waits) > 1:
                    changed = True
                    for wx in waits[:-1]:
                        nop = mybir.InstNoOp(name=f"wsplit-{cnt}", ins=[], outs=[])
                        cnt += 1
                        nop.engine = ins.engine
                        nop.sync_info = mybir.SyncInfo(on_wait=[wx], on_update=[])
                        nc.register_instruction(nop, overwrite=True)
                        out.append(nop)
                    si.on_wait = [waits[-1]]
                out.append(ins)
            if changed:
                bb.instructions = out


def _emit(ctx, nc, tc, t_in, t_out):
    x_d = t_in["x"]

    singles = ctx.enter_context(tc.tile_pool(name="singles", bufs=1))
    xtp = ctx.enter_context(tc.tile_pool(name="xtp", bufs=4))
    fgp = ctx.enter_context(tc.tile_pool(name="fgp", bufs=2))
    etp = ctx.enter_context(tc.tile_pool(name="etp", bufs=8))
    work = ctx.enter_context(tc.tile_pool(name="work", bufs=4))

    # ---------------- constants (Pool queue head: nothing may block them) --
    identity_f = singles.tile([128, 128], FP)
    make_identity(nc, identity_f[:])
    identity = singles.tile([128, 128], FPR)
    nc.vector.tensor_copy(out=identity[:], in_=identity_f[:])
    # rep_two[d, 32t + d'] = (d % 32 == d'), d in [0, 64): replicates a
    # 32-row tile into all 4 PE row groups via one matmul. Rows 0:32 serve
    # the f half (base partition 0), rows 32:64 the g half (base 32).
    rep_two_f = singles.tile([2 * CK, 128], FP)
    nc.gpsimd.memset(rep_two_f[:], 0.0)
    for half in range(2):
        nc.gpsimd.affine_select(
            out=rep_two_f[:].rearrange("d (t c) -> d t c", t=4),
            in_=rep_two_f[:].rearrange("d (t c) -> d t c", t=4),
            compare_op=ALU.not_equal,
            fill=1.0,
            base=-CK * half,
            pattern=[[0, 4], [-1, CK]],
            channel_multiplier=1,
        )
    rep_two = singles.tile([2 * CK, 128], FPR)
    nc.vector.tensor_copy(out=rep_two[:], in_=rep_two_f[:])
    ones_col = singles.tile([128, 1], BF)
    ones_f = singles.tile([128, 1], FP)
    nc.vector.memset(ones_f[:], 1.0)
    nc.vector.tensor_copy(out=ones_col[:], in_=ones_f[:])
    ones_row_f = singles.tile([1, 128], FP)
    nc.vector.memset(ones_row_f[:], 1.0)
    ones_row = singles.tile([1, 128], FPR)
    nc.vector.tensor_copy(out=ones_row[:], in_=ones_row_f[:])

    # ---------------- input DMAs (SP/ACT queues share HWDGE; gpsimd SWDGE) -
    x_view = x_d.ap().rearrange("(t p) c -> p t c", p=128)
    out_view = t_out.ap().rearrange("(t p) c -> p t c", p=128)
    x_pix = singles.tile([128, 32, C], FPR)
    # first chunk split so the first transposes start sooner
    nc.sync.dma_start(out=x_pix[:, 0:1, :], in_=x_view[:, 0:1, :].bitcast(FPR))
    nc.sync.dma_start(out=x_pix[:, 1:2, :], in_=x_view[:, 1:2, :].bitcast(FPR))
    nc.sync.dma_start(out=x_pix[:, 2:4, :], in_=x_view[:, 2:4, :].bitcast(FPR))

    wh_sb = singles.tile([128, 2, C], FPR)
    wo_sb = singles.tile([128, 2, C], FPR)
    wh_v = t_in["Wh"].ap().rearrange("(kc p) c -> p kc c", p=128)
    wo_v = t_in["Wo"].ap().rearrange("(kc p) c -> p kc c", p=128)
    for s in range(1, 3):
        nc.sync.dma_start(out=x_pix[:, 4 * s:4 * (s + 1), :], in_=x_view[:, 4 * s:4 * (s + 1), :].bitcast(FPR))
    nc.sync.dma_start(out=wh_sb[:], in_=wh_v.bitcast(FPR))
    nc.sync.dma_start(out=wo_sb[:], in_=wo_v.bitcast(FPR))
    nc.sync.dma_start(out=x_pix[:, 12:16, :], in_=x_view[:, 12:16, :].bitcast(FPR))

    # small tensors + far-tail x chunks on the gpsimd SWDGE path (bypasses
    # HWDGE and the SP/ACT sequencers)
    wfg_sb = singles.tile([128, 2, 2 * CK], FPR)
    wf_v = t_in["Wf"].ap().rearrange("(kc p) d -> p kc d", p=128)
    wg_v = t_in["Wg"].ap().rearrange("(kc p) d -> p kc d", p=128)
    nc.scalar.dma_start(out=wfg_sb[:, :, 0:CK], in_=wf_v.bitcast(FPR))
    nc.scalar.dma_start(out=wfg_sb[:, :, CK:2 * CK], in_=wg_v.bitcast(FPR))

    bfg_rep = singles.tile([2 * CK, 1], FP)
    nc.gpsimd.dma_start(out=bfg_rep[0:CK, :], in_=t_in["bf"][:].unsqueeze(1))
    nc.gpsimd.dma_start(out=bfg_rep[CK:2 * CK, :], in_=t_in["bg"][:].unsqueeze(1))

    bh_col = singles.tile([128, 2, 1], FPR)
    bh_v = t_in["bh"].ap().rearrange("(kc p) -> p kc", p=128).unsqueeze(2)
    nc.gpsimd.dma_start(out=bh_col[:], in_=bh_v.bitcast(FPR))
    bo_row = singles.tile([1, C], FP)
    nc.gpsimd.dma_start(out=bo_row[0:1, :], in_=t_in["bo"][:].unsqueeze(0))

    for s in range(4, 8):
        nc.gpsimd.dma_start(out=x_pix[:, 4 * s:4 * (s + 1), :], in_=x_view[:, 4 * s:4 * (s + 1), :].bitcast(FPR))

    gamma_st = singles.tile([1, 1], FPR)
    nc.gpsimd.dma_start(out=gamma_st[0:1, :], in_=t_in["gamma"][:].unsqueeze(0).bitcast(FPR))
    gamma_rep = singles.tile([128, 1], FP)

    # ---------------- prologue psum pools ---------------------------------
    pre_ctx = ExitStack()
    ps_tr = pre_ctx.enter_context(tc.tile_pool(name="ps_tr", bufs=3, space="PSUM"))
    ps_fg = pre_ctx.enter_context(tc.tile_pool(name="ps_fg", bufs=2, space="PSUM"))
    ps_hw = pre_ctx.enter_context(tc.tile_pool(name="ps_hw", bufs=2, space="PSUM"))

    # ---- fused output projection: Whw = Wh @ Wo, bhw = bh @ Wo + bo.
    # Emitted after slice 0's projections so the PE queue head doesn't block
    # on the (later-arriving) Wh/Wo DMAs.
    whw_sb = singles.tile([128, 2, C], FPR)  # [i % 128, i // 128, o]
    bhw_bc2 = singles.tile([128, 2, C], FP)

    def emit_whw():
        whT = singles.tile([128, 2, C], FPR)  # [m % 128, m // 128, i] = Wh[i, m]
        for mc in range(2):
            for ib in range(2):
                pt = ps_tr.tile([128, 128], FPR, tag="trw", bufs=2, name=f"ptw_{mc}_{ib}")
                nc.tensor.transpose(
                    pt[:], wh_sb[:, ib, 128 * mc:128 * (mc + 1)],
                    identity[:],
                )
                nc.vector.tensor_copy(out=whT[:, mc, 128 * ib:128 * (ib + 1)], in_=pt[:])

        for ib in range(2):
            ps = ps_fg.tile([128, C], FP, tag="fg", name=f"psw{ib}")
            for mc in range(2):
                nc.tensor.matmul(
                    ps[:],
                    whT[:, mc, 128 * ib:128 * (ib + 1)],
                    wo_sb[:, mc, :],
                    start=(mc == 0),
                    stop=(mc == 1),
                )
            nc.vector.tensor_copy(out=whw_sb[:, ib, :], in_=ps[:])

        # bhw = bh @ Wo + bo, in pair layout for the hw1 adds
        ps_b = ps_hw.tile([1, C], FP, tag="hw1", name="ps_b")
        for kc in range(2):
            nc.tensor.matmul(
                ps_b[:], bh_col[:, kc, :], wo_sb[:, kc, :], start=(kc == 0), stop=(kc == 1)
            )
        bhw_st = singles.tile([1, 2, C], FPR)
        nc.vector.tensor_add(out=bhw_st[:, 0, :], in0=ps_b[:], in1=bo_row[:])
        nc.vector.tensor_copy(out=bhw_st[:, 1, :], in_=bhw_st[:, 0, :])
        ps_bb = ps_hw.tile([128, 2, C], FP, tag="hw1", name="ps_bb")
        nc.tensor.matmul(ps_bb[:], ones_row[:], bhw_st[:], start=True, stop=True)
        nc.vector.tensor_copy(out=bhw_bc2[:], in_=ps_bb[:])

    # ---------------- per-slice prologue pipeline -------------------------
    # slice s owns pixels [512s, 512(s+1)) = pixel blocks 4s+t; attention
    # iteration g consumes key blocks {4g+t} (row group t) and hw blocks 4g+t.
    ft_rep = []  # [s] -> [128, 512] f^T replicated into 4 row groups
    gt4 = singles.tile([128, N], FPR)  # g^T replicated, all queries
    hw1 = singles.tile([128, 32, C], BF)  # pixel-major x @ Whw + bhw

    xts = {}

    def emit_tr(s):
        xt = xtp.tile([128, 2, 512], FPR, tag="xt", name=f"xt{s}")
        xts[s] = xt
        for kc in range(2):
            trt = ps_tr.tile([128, 4, 128], FPR, tag="tr", bufs=2, name=f"tr_{s}_{kc}")
            for blk in range(4):
                nc.tensor.transpose(
                    trt[:, blk, :],
                    x_pix[:, 4 * s + blk, 128 * kc:128 * (kc + 1)],
                    identity[:],
                )
            if kc == 0:
                nc.vector.tensor_copy(out=xt[:, kc, :], in_=trt[:].rearrange("p a b -> p (a b)"))
            else:
                nc.scalar.activation(out=xt[:, kc, :], in_=trt[:].rearrange("p a b -> p (a b)"),
                                     func=AF.Identity, bias=0.0)

    def emit_proj(s):
        # f/g projections together: psfg[d, key] (f rows 0:32, g rows 32:64)
        xt = xts[s]
        psfg = ps_fg.tile([2 * CK, 512], FP, tag="fg", name=f"psfg{s}")
        for kc in range(2):
            nc.tensor.matmul(
                psfg[:], wfg_sb[:, kc, :], xt[:, kc, :], start=(kc == 0), stop=(kc == 1)
            )
        fg_sb = fgp.tile([2 * CK, 512], FPR, tag="fgsb", name=f"fg_sb{s}")
        nc.scalar.activation(
            out=fg_sb[:], in_=psfg[:], func=AF.Identity, bias=bfg_rep[:]
        )

        # replicate f^T / g^T into all 4 row groups with REP matmuls
        psrf = ps_fg.tile([128, 512], FP, tag="fg", name=f"psrf{s}")
        nc.tensor.matmul(psrf[:], rep_two[0:CK, :], fg_sb[0:CK, :], start=True, stop=True)
        ftr = singles.tile([128, 512], FPR, name=f"ftr{s}")
        nc.scalar.activation(out=ftr[:], in_=psrf[:], func=AF.Identity, bias=0.0)
        ft_rep.append(ftr)
        psrg = ps_fg.tile([128, 512], FP, tag="fg", name=f"psrg{s}")
        nc.tensor.matmul(psrg[:], rep_two[CK:2 * CK, :], fg_sb[CK:2 * CK, :], start=True, stop=True)
        nc.scalar.activation(out=gt4[:, 512 * s:512 * (s + 1)], in_=psrg[:], func=AF.Identity, bias=0.0)

    def emit_hw1(s):
        # hw1 blocks for this slice (pairs share a PSUM bank)
        xt = xts.pop(s)
        for half in range(2):
            hps = ps_hw.tile([128, 2, C], FP, tag="hw1", name=f"hps_{s}_{half}")
            for b2 in range(2):
                blk = 2 * half + b2
                for kc in range(2):
                    nc.tensor.matmul(
                        hps[:, b2, :],
                        xt[:, kc, 128 * blk:128 * (blk + 1)],
                        whw_sb[:, kc, :],
                        start=(kc == 0),
                        stop=(kc == 1),
                    )
            nc.vector.tensor_add(
                out=hw1[:, 4 * s + 2 * half:4 * s + 2 * half + 2, :],
                in0=hps[:],
                in1=bhw_bc2[:],
            )

    # software-pipelined emission: transposes run two slices ahead of the
    # projection chain (which waits on ACT/DVE copies), hw1 two behind it.
    emit_tr(0)
    emit_tr(1)
    emit_tr(2)
    emit_proj(0)
    emit_whw()
    emit_tr(3)
    emit_hw1(0)
    emit_proj(1)
    for s in range(4, 8):
        emit_tr(s)
        emit_hw1(s - 3)
        emit_proj(s - 2)
    emit_hw1(5)
    emit_proj(6)
    emit_hw1(6)
    emit_proj(7)
    emit_hw1(7)

    misc_st = singles.tile([1, 128], FPR)
    nc.vector.memset(misc_st[:].bitcast(FP), 0.0)
    nc.vector.tensor_copy(out=misc_st[0:1, 0:1], in_=gamma_st[0:1, :])
    ps_gm = ps_fg.tile([128, 128], FP, tag="fg", name="ps_gm")
    nc.tensor.matmul(ps_gm[:], ones_row[:], misc_st[:], start=True, stop=True)
    nc.vector.tensor_copy(out=gamma_rep[:], in_=ps_gm[:, 0:1])
    pre_ctx.close()

    # ---------------- attention -------------------------------------------
    ps_sc = ctx.enter_context(tc.tile_pool(name="ps_sc", bufs=4, space="PSUM"))
    ps_o = ctx.enter_context(tc.tile_pool(name="ps_o", bufs=3, space="PSUM"))
    ps_sum = ctx.enter_context(tc.tile_pool(name="ps_sum", bufs=1, space="PSUM"))

    sums_ps = ps_sum.tile([128, 32], FP, name="sums")

    o_tiles = {}
    et_tiles = {}

    def emit_scores(k):
        qs, g = divmod(k, 8)
        et2 = []
        for half in range(2):
            sc = ps_sc.tile([128, 1024], FP, tag="score", bufs=2,
                            name=f"sc_{qs}_{g}_{half}")
            for t2 in range(2):
                t = 2 * half + t2
                nc.tensor.matmul(
                    sc[:, 512 * t2:512 * (t2 + 1)],
                    ft_rep[g][32 * t:32 * (t + 1), 128 * t:128 * (t + 1)],
                    gt4[32 * t:32 * (t + 1), 512 * qs:512 * (qs + 1)],
                    start=True,
                    stop=True,
                    tile_position=(32 * t, 0),
                )
            e = etp.tile([128, 1024], BF, tag="et", name=f"et_{qs}_{g}_{half}")
            nc.scalar.activation(out=e[:], in_=sc[:], func=AF.Exp)
            et2.append(e)
        et_tiles[k] = et2

    def emit_oacc(k):
        qs, g = divmod(k, 8)
        if g == 0:
            o_tiles[qs] = [
                ps_o.tile([128, 512], FP, tag="oacc", name=f"oacc_{qs}_{h}")
                for h in range(2)
            ]
        o_ps = o_tiles[qs]
        et2 = et_tiles.pop(k)
        et = [et2[t // 2][:, 512 * (t % 2):512 * (t % 2 + 1)] for t in range(4)]
        last_g = g == 7
        for t in range(4):
            kb = 4 * g + t
            last = last_g and t == 3
            first = g == 0 and t == 0
            # one accumulation group per PSUM tile: start on the tile's first
            # write, stop on its last (has_written bits cover the rest)
            for j in range(4):
                nc.tensor.matmul(
                    sums_ps[:, 4 * qs + j:4 * qs + j + 1],
                    et[t][:, 128 * j:128 * (j + 1)],
                    ones_col[:],
                    start=first and j == 0,
                    stop=last and j == 3,
                )
            for j in range(4):
                nc.tensor.matmul(
                    o_ps[j // 2][:, 256 * (j % 2):256 * (j % 2 + 1)],
                    et[t][:, 128 * j:128 * (j + 1)],
                    hw1[:, kb, :],
                    start=first and j % 2 == 0,
                    stop=last and j % 2 == 1,
                )

    # software-pipelined: scores/exps of iteration k+1 are emitted (and thus
    # prioritized) ahead of iteration k's o accumulation.
    emit_scores(0)
    for k in range(64):
        if k + 1 < 64:
            emit_scores(k + 1)
        emit_oacc(k)
        if k % 8 != 7:
            continue
        qs = k // 8
        o_ps = o_tiles.pop(qs)
        # epilogue: out = gamma/rowsum * o + x, per 128-pixel block
        for h in range(2):
            out_sb = work.tile([128, 2, C], FP, tag="outsb", name=f"osb_{qs}_{h}")
            for b2 in range(2):
                j = 2 * h + b2
                blk = 4 * qs + j
                rinv = work.tile([128, 1], FP, tag="rinv", name=f"rinv_{blk}")
                nc.vector.reciprocal(out=rinv[:], in_=sums_ps[:, 4 * qs + j:4 * qs + j + 1])
                nc.vector.tensor_mul(out=rinv[:], in0=rinv[:], in1=gamma_rep[:])
                nc.vector.scalar_tensor_tensor(
                    out=out_sb[:, b2, :],
                    in0=o_ps[h][:, 256 * b2:256 * (b2 + 1)],
                    scalar=rinv[:],
                    in1=x_pix[:, blk, :].bitcast(FP),
                    op0=ALU.mult,
                    op1=ALU.add,
                )
            blk0 = 4 * qs + 2 * h
            nc.sync.dma_start(
                out=out_view[:, blk0:blk0 + 2, :],
                in_=out_sb[:],
            )


_CACHE = {}


def _build():
    if "nc" not in _CACHE:
        nc = bass.Bass("TRN2", target_bir_lowering=False, debug=False)
        t_in = {
            "x": nc.dram_tensor("x", [N, C], FP, kind="ExternalInput"),
            "Wf": nc.dram_tensor("Wf", [C, CK], FP, kind="ExternalInput"),
            "bf": nc.dram_tensor("bf", [CK], FP, kind="ExternalInput"),
            "Wg": nc.dram_tensor("Wg", [C, CK], FP, kind="ExternalInput"),
            "bg": nc.dram_tensor("bg", [CK], FP, kind="ExternalInput"),
            "Wh": nc.dram_tensor("Wh", [C, C], FP, kind="ExternalInput"),
            "bh": nc.dram_tensor("bh", [C], FP, kind="ExternalInput"),
            "Wo": nc.dram_tensor("Wo", [C, C], FP, kind="ExternalInput"),
            "bo": nc.dram_tensor("bo", [C], FP, kind="ExternalInput"),
            "gamma": nc.dram_tensor("gamma", [1], FP, kind="ExternalInput"),
        }
        t_out = nc.dram_tensor("out", [N, C], FP, kind="ExternalOutput")
        with tile.TileContext(nc) as tc:
            with ExitStack() as ctx:
                _emit(ctx, nc, tc, t_in, t_out)
        _split_instruction_waits(nc)
        _CACHE["nc"] = nc
    return _CACHE["nc"]


def kernel(x, Wf, bf, Wg, bg, Wh, bh, Wo, bo, gamma, _trace=False, _tmpdir=None):
    nc = _build()
    x = np.ascontiguousarray(np.asarray(x, dtype=np.float32)).reshape(B, N, C)
    w = {
        "Wf": np.ascontiguousarray(np.asarray(Wf, np.float32)),
        "bf": np.ascontiguousarray(np.asarray(bf, np.float32)),
        "Wg": np.ascontiguousarray(np.asarray(Wg, np.float32)),
        "bg": np.ascontiguousarray(np.asarray(bg, np.float32)),
        "Wh": np.ascontiguousarray(np.asarray(Wh, np.float32)),
        "bh": np.ascontiguousarray(np.asarray(bh, np.float32)),
        "Wo": np.ascontiguousarray(np.asarray(Wo, np.float32)),
        "bo": np.ascontiguousarray(np.asarray(bo, np.float32)),
        "gamma": np.ascontiguousarray(np.asarray(gamma, np.float32)),
    }
    in_maps = [dict(w, x=x[i]) for i in range(NCORES)]
    res = run_bass_kernel_spmd(
        nc, in_maps, core_ids=list(range(NCORES)), trace=_trace, tmpdir=_tmpdir
    )
    out = np.stack([res.results[i]["out"] for i in range(NCORES)])
    if _trace:
        kernel._last_result = res
    return out.reshape(B, H, W, C).astype(np.float32)
